# revision 1
# baseline (speedup 1.0000x reference)
"""Trainium2 Bass kernel for nn_CustomTransformer_58445914964311.

12-layer MoE transformer (768 embd, 8 heads, 8 experts top-2, B=8 x T=64
tokens), distributed over 8 NeuronCores:
  - attention sharded by head (core c computes head c for all 512 tokens),
  - MoE sharded by expert (core c computes expert c densely for all tokens,
    weighted by its combine weight),
  - per-layer AllReduce combines the per-head attention partials and the
    per-expert MoE partials; everything else is computed replicated.

All activations are kept feature-major in SBUF ([features-on-partitions,
tokens-on-free]) so linear layers chain on the PE without transposes. All
matmuls run in plain fp32 (float32r / bf16 shift the top-2 gate decisions
vs the fp32 reference, which costs whole-token errors).

Self-contained: hardcodes all shapes; host side only reshapes/transposes
and shards the incoming fp32 weights.
"""

import numpy as np

import concourse.bass as bass
import concourse.mybir as mybir
import concourse.tile as tile
from concourse.bass_utils import run_bass_kernel_spmd

import os
import sys

# ---------------------------------------------------------------------------
# Compatibility patches (inlined): the walrus build here rejects instructions
# carrying more than one semaphore wait ("Too many sync wait commands").
# 1) split the Tile kernel-tail drain's waits onto separate sync nops;
# 2) post-process the serialized BIR, peeling extra waits onto injected
#    EventSemaphore instructions;
# 3) recreate the missing antenv.axon_hooks registry so trace=True works.
# ---------------------------------------------------------------------------
import orjson as _orjson
from concourse.vector_clock import ScopedClock as _ScopedClock

_COMPAT_DONE = False


def _patched_drain_and_barrier(self, tick_clock, wait_clock):
    nc = self.nc
    collector = nc.sync.nop()
    wait_clock.add_sem_waits(
        collector.ins, _ScopedClock({None: tick_clock.global_clock})
    )
    si = collector.ins.sync_info
    waits = list(si.on_wait or []) if si is not None else []
    if len(waits) > 1:
        si.on_wait = waits[:1]
        for w in waits[1:]:
            extra = nc.sync.nop()
            esi = extra.ins.sync_info
            if esi is None:
                extra.ins.sync_info = mybir.SyncInfo(on_wait=[w], on_update=[])
            else:
                esi.on_wait = [w]
    nc.sync.drain()
    nc.all_engine_barrier()
    popped = nc._tile_sem_poison_stack.pop()
    assert popped is self._sem_poison
    nc.clear_and_free_semaphores(list(self.sems.allocated().values()))
    nc.all_engine_barrier()


def _split_multi_waits(mod, max_waits=1):
    ctr = 0
    for fn in mod.get("functions", []):
        for blk in fn.get("blocks", []):
            insts = blk.get("instructions", [])
            if not any(
                len((i.get("sync_info") or {}).get("on_wait") or []) > max_waits
                for i in insts
            ):
                continue
            new_insts = []
            for inst in insts:
                si = inst.get("sync_info")
                waits = (si.get("on_wait") or []) if si else []
                if len(waits) > max_waits:
                    for w in waits[max_waits:]:
                        ctr += 1
                        new_insts.append({
                            "debug": inst.get("debug", 0),
                            "engine": inst["engine"],
                            "ins": [], "outs": [],
                            "name": f"{inst['name']}-wsp{ctr}",
                            "opcode": "EventSemaphore",
                            "sync_info": {"on_update": [], "on_wait": [w]},
                        })
                    si["on_wait"] = waits[:max_waits]
                new_insts.append(inst)
            blk["instructions"] = new_insts
    return mod


_orig_to_json_bytes = bass.Bass.to_json_bytes


def _patched_to_json_bytes(self):
    return _orjson.dumps(_split_multi_waits(_orjson.loads(_orig_to_json_bytes(self))))


def _install_ntff_hook_shim():
    import types
    if "antenv.axon_hooks" in sys.modules:
        return
    try:
        import antenv  # noqa: F401
    except ImportError:
        return
    mod = types.ModuleType("antenv.axon_hooks")
    _state = {"hook": None}
    mod.set_axon_ntff_profile_hook = lambda hook: _state.__setitem__("hook", hook)
    mod.get_axon_ntff_profile_hook = lambda: _state["hook"]
    sys.modules["antenv.axon_hooks"] = mod
    sys.modules["antenv"].axon_hooks = mod
    try:
        from trn_agent_boot.trn_boot import _ntff_profile_via_ctypes
        hook = _ntff_profile_via_ctypes("/opt/axon/libaxon_pjrt.so")
        if hook is not None:
            mod.set_axon_ntff_profile_hook(hook)
    except Exception:
        pass


def _install_compat():
    global _COMPAT_DONE
    if _COMPAT_DONE:
        return
    tile.TileContext._drain_and_barrier = _patched_drain_and_barrier
    bass.Bass.to_json_bytes = _patched_to_json_bytes
    _install_ntff_hook_shim()
    _COMPAT_DONE = True


_install_compat()

F32 = mybir.dt.float32
I32 = mybir.dt.int32
AF = mybir.ActivationFunctionType
ALU = mybir.AluOpType
AX = mybir.AxisListType

N_CORES = 8
L = 12
D = 768
H = 96          # head dim
NH = 8
E = 8           # experts
DFF = 3072
B, T = 8, 64
N = B * T       # 512 tokens
V = 99
KT = D // 128   # 6 feature tiles
MT = DFF // 128  # 24 dff tiles
EPS = 1e-5
SCALE = H ** -0.5

_CACHED = {}


def build():
    nc = bass.Bass(num_devices=N_CORES)

    # ---- inputs (per-core data, same names) ----
    d_idx = nc.dram_tensor("idx", [1, N], I32, kind="ExternalInput")
    d_iota = nc.dram_tensor("iota99", [V, 1], F32, kind="ExternalInput")
    d_ident = nc.dram_tensor("ident128", [128, 128], F32, kind="ExternalInput")
    d_mask = nc.dram_tensor("maskb", [64, 64], F32, kind="ExternalInput")
    d_ones_col = nc.dram_tensor("ones_col", [128, 1], F32, kind="ExternalInput")
    d_ones_row = nc.dram_tensor("ones_row", [1, 128], F32, kind="ExternalInput")
    d_tok = nc.dram_tensor("tok_emb", [V, D], F32, kind="ExternalInput")
    d_posT = nc.dram_tensor("posT", [D, N], F32, kind="ExternalInput")
    d_wqT = nc.dram_tensor("wqT", [L, KT, 128, H], F32, kind="ExternalInput")
    d_wkT = nc.dram_tensor("wkT", [L, KT, 128, H], F32, kind="ExternalInput")
    d_wvT = nc.dram_tensor("wvT", [L, KT, 128, H], F32, kind="ExternalInput")
    d_wpT = nc.dram_tensor("wpT", [L, H, D], F32, kind="ExternalInput")
    d_bproj = nc.dram_tensor("bproj", [L, KT, 128], F32, kind="ExternalInput")
    d_gwT = nc.dram_tensor("gwT", [L, KT, 128, E], F32, kind="ExternalInput")
    d_gb = nc.dram_tensor("gb", [L, 1, E], F32, kind="ExternalInput")
    d_w1T = nc.dram_tensor("w1T", [L, MT, KT, 128, 128], F32, kind="ExternalInput")
    d_b1 = nc.dram_tensor("b1", [L, MT, 128], F32, kind="ExternalInput")
    d_w2T = nc.dram_tensor("w2T", [L, KT, MT, 128, 128], F32, kind="ExternalInput")
    d_b2 = nc.dram_tensor("b2all", [L, E, D], F32, kind="ExternalInput")
    d_combsel = nc.dram_tensor("combsel", [E, 1], F32, kind="ExternalInput")
    d_ln1w = nc.dram_tensor("ln1w", [L, KT, 128], F32, kind="ExternalInput")
    d_ln1b = nc.dram_tensor("ln1b", [L, KT, 128], F32, kind="ExternalInput")
    d_ln2w = nc.dram_tensor("ln2w", [L, KT, 128], F32, kind="ExternalInput")
    d_ln2b = nc.dram_tensor("ln2b", [L, KT, 128], F32, kind="ExternalInput")
    d_lnfw = nc.dram_tensor("lnfw", [KT, 128], F32, kind="ExternalInput")
    d_lnfb = nc.dram_tensor("lnfb", [KT, 128], F32, kind="ExternalInput")
    d_lmT = nc.dram_tensor("lmT", [KT, 128, V], F32, kind="ExternalInput")
    d_lmb = nc.dram_tensor("lmb", [V, 1], F32, kind="ExternalInput")
    d_out = nc.dram_tensor("logitsT", [V, N], F32, kind="ExternalOutput")

    with tile.TileContext(nc) as tc:
        with (
            tc.tile_pool(name="const", bufs=1) as cpool,
            tc.tile_pool(name="x", bufs=1) as xpool,
            tc.tile_pool(name="attw", bufs=1) as awpool,
            tc.tile_pool(name="w1", bufs=3) as w1pool,
            tc.tile_pool(name="w2", bufs=2) as w2pool,
            tc.tile_pool(name="h", bufs=1) as hpool,
            tc.tile_pool(name="work", bufs=2) as wk,
            tc.tile_pool(name="small", bufs=3) as sm,
            tc.tile_pool(name="ps_acc", bufs=3, space="PSUM") as ps_acc,
            tc.tile_pool(name="ps_small", bufs=3, space="PSUM") as ps_small,
            tc.tile_pool(name="ps_bc", bufs=2, space="PSUM") as ps_bc,
            tc.tile_pool(name="dram", bufs=1, space="DRAM") as dpool,
        ):
            # ---- constants resident ----
            ident = cpool.tile([128, 128], F32, name="ident")
            nc.sync.dma_start(ident[:], d_ident[:])
            maskb = cpool.tile([64, 64], F32, name="maskb")
            nc.sync.dma_start(maskb[:], d_mask[:])
            iota99 = cpool.tile([V, 1], F32, name="iota99")
            nc.sync.dma_start(iota99[:], d_iota[:])
            ones_col = cpool.tile([128, 1], F32, name="ones_col")
            nc.sync.dma_start(ones_col[:], d_ones_col[:])
            ones_row = cpool.tile([1, 128], F32, name="ones_row")
            nc.sync.dma_start(ones_row[:], d_ones_row[:])
            combsel = cpool.tile([E, 1], F32, name="combsel")
            nc.sync.dma_start(combsel[:], d_combsel[:])
            tok = cpool.tile([V, D], F32, name="tok")
            nc.sync.dma_start(tok[:], d_tok[:])
            posT = wk.tile([128, KT * N], F32, name="ln_t", bufs=1)
            for k in range(KT):
                nc.sync.dma_start(posT[:, k * N:(k + 1) * N],
                                  d_posT[k * 128:(k + 1) * 128, :])
            lmT = cpool.tile([128, KT * V], F32, name="lmT")
            for k in range(KT):
                nc.sync.dma_start(lmT[:, k * V:(k + 1) * V], d_lmT[k])
            lmb = cpool.tile([V, 1], F32, name="lmb")
            nc.sync.dma_start(lmb[:], d_lmb[:])
            lnfw = cpool.tile([128, KT], F32, name="lnfw")
            nc.sync.dma_start(lnfw[:], d_lnfw.rearrange("a p -> p a"))
            lnfb = cpool.tile([128, KT], F32, name="lnfb")
            nc.sync.dma_start(lnfb[:], d_lnfb.rearrange("a p -> p a"))

            # AR bounce tensors
            ar_ins = [dpool.tile([D, N], F32, name=f"ari{i}") for i in range(2 * L)]
            ar_outs = [dpool.tile([D, N], F32, name=f"aro{i}",
                                  addr_space="Shared") for i in range(2 * L)]

            # ---- x state: 6 tiles [128, N] ----
            x_sb = xpool.tile([128, KT * N], F32, name="x_sb")

            def xs(k):
                return x_sb[:, k * N:(k + 1) * N]

            # ---- embedding ----
            idx_i = sm.tile([1, N], I32, name="idx_i", bufs=1)
            nc.sync.dma_start(idx_i[:], d_idx[:])
            idx_f = sm.tile([1, N], F32, name="idx_f", bufs=1)
            nc.vector.tensor_copy(idx_f[:], idx_i[:])
            idxbc = ps_bc.tile([V, N], F32, tag="bc")
            nc.tensor.matmul(idxbc[:], ones_row[:, :V], idx_f[:],
                             start=True, stop=True)
            onehot = wk.tile([V, N], F32, name="onehot", bufs=1)
            nc.vector.tensor_scalar(onehot[:], idxbc[:], iota99[:], None,
                                    op0=ALU.is_equal)
            for k in range(KT):
                e_ps = ps_acc.tile([128, N], F32, tag="acc")
                nc.tensor.matmul(e_ps[:], tok[:, k * 128:(k + 1) * 128],
                                 onehot[:], start=True, stop=True)
                nc.vector.tensor_add(xs(k), e_ps[:], posT[:, k * N:(k + 1) * N])

            def layernorm(get_t, w_ap, b_ap, extra_ps=None):
                """get_t(k) -> [128, N] AP of pre-norm values (read twice).
                Writes normalized result into x_sb. extra_ps: optional psum
                [128,N]-AP-producing fn added into t (used for b2@combT)."""
                s_ps = ps_small.tile([1, N], F32, tag="sm")
                q_ps = ps_small.tile([1, N], F32, tag="sm")
                tmp = wk.tile([128, KT * N], F32, name="ln_t", bufs=1)
                for k in range(KT):
                    tk = tmp[:, k * N:(k + 1) * N]
                    src = get_t(k)
                    if extra_ps is not None:
                        nc.vector.tensor_add(tk, src, extra_ps(k))
                        src = tk
                    else:
                        nc.vector.tensor_copy(tk, src)
                    sq = sm.tile([128, N], F32, tag="lnsq", bufs=2)
                    nc.scalar.activation(sq[:], tk, AF.Square)
                    nc.tensor.matmul(s_ps[:], ones_col[:], tk,
                                     start=(k == 0), stop=(k == KT - 1))
                    nc.tensor.matmul(q_ps[:], ones_col[:], sq[:],
                                     start=(k == 0), stop=(k == KT - 1))
                mu = sm.tile([1, N], F32, tag="ln1", bufs=1)
                nc.vector.tensor_scalar_mul(mu[:], s_ps[:], 1.0 / D)
                mu2 = sm.tile([1, N], F32, tag="ln2", bufs=1)
                nc.vector.tensor_mul(mu2[:], mu[:], mu[:])
                var = sm.tile([1, N], F32, tag="ln3", bufs=1)
                nc.vector.scalar_tensor_tensor(var[:], q_ps[:], 1.0 / D, mu2[:],
                                               op0=ALU.mult, op1=ALU.subtract)
                nc.vector.tensor_scalar_add(var[:], var[:], EPS)
                sd = sm.tile([1, N], F32, tag="ln4", bufs=1)
                nc.scalar.activation(sd[:], var[:], AF.Sqrt)
                rstd = sm.tile([1, N], F32, tag="ln5", bufs=1)
                nc.vector.reciprocal(rstd[:], sd[:])
                nmu = sm.tile([1, N], F32, tag="ln6", bufs=1)
                nc.vector.tensor_scalar_mul(nmu[:], mu[:], -1.0)
                nmu_bc = ps_bc.tile([128, N], F32, tag="bc")
                nc.tensor.matmul(nmu_bc[:], ones_row[:], nmu[:],
                                 start=True, stop=True)
                rstd_bc = ps_bc.tile([128, N], F32, tag="bc")
                nc.tensor.matmul(rstd_bc[:], ones_row[:], rstd[:],
                                 start=True, stop=True)
                for k in range(KT):
                    tk = tmp[:, k * N:(k + 1) * N]
                    u = sm.tile([128, N], F32, tag="lnu", bufs=2)
                    nc.vector.tensor_add(u[:], tk, nmu_bc[:])
                    nc.vector.tensor_mul(u[:], u[:], rstd_bc[:])
                    nc.vector.tensor_scalar(xs(k), u[:], w_ap[:, k:k + 1],
                                            b_ap[:, k:k + 1],
                                            op0=ALU.mult, op1=ALU.add)

            for l in range(L):
                # ---- layer weights ----
                wq = awpool.tile([128, KT * H], F32, tag="wq")
                wkk = awpool.tile([128, KT * H], F32, tag="wk")
                wv = awpool.tile([128, KT * H], F32, tag="wv")
                for k in range(KT):
                    nc.sync.dma_start(wq[:, k * H:(k + 1) * H], d_wqT[l, k])
                    nc.sync.dma_start(wkk[:, k * H:(k + 1) * H], d_wkT[l, k])
                    nc.sync.dma_start(wv[:, k * H:(k + 1) * H], d_wvT[l, k])
                wp = awpool.tile([H, D], F32, tag="wp")
                nc.sync.dma_start(wp[:], d_wpT[l])
                bpj = awpool.tile([128, KT], F32, tag="bpj")
                nc.sync.dma_start(bpj[:], d_bproj[l].rearrange("a p -> p a"))
                gw = awpool.tile([128, KT * E], F32, tag="gw")
                for k in range(KT):
                    nc.sync.dma_start(gw[:, k * E:(k + 1) * E], d_gwT[l, k])
                gb = awpool.tile([1, E], F32, tag="gb")
                nc.sync.dma_start(gb[:], d_gb[l])
                l1w = awpool.tile([128, KT], F32, tag="l1w")
                nc.sync.dma_start(l1w[:], d_ln1w[l].rearrange("a p -> p a"))
                l1b = awpool.tile([128, KT], F32, tag="l1b")
                nc.sync.dma_start(l1b[:], d_ln1b[l].rearrange("a p -> p a"))
                l2w = awpool.tile([128, KT], F32, tag="l2w")
                nc.sync.dma_start(l2w[:], d_ln2w[l].rearrange("a p -> p a"))
                l2b = awpool.tile([128, KT], F32, tag="l2b")
                nc.sync.dma_start(l2b[:], d_ln2b[l].rearrange("a p -> p a"))
                b1t = awpool.tile([128, MT], F32, tag="b1t")
                nc.sync.dma_start(b1t[:], d_b1[l].rearrange("a p -> p a"))
                b2t = awpool.tile([E, D], F32, tag="b2t")
                nc.sync.dma_start(b2t[:], d_b2[l])

                # ---- attention: this core's head ----
                q_ps = ps_acc.tile([H, N], F32, tag="acc")
                k_ps = ps_acc.tile([H, N], F32, tag="acc")
                v_ps = ps_acc.tile([H, N], F32, tag="acc")
                for k in range(KT):
                    nc.tensor.matmul(q_ps[:], wq[:, k * H:(k + 1) * H], xs(k),
                                     start=(k == 0), stop=(k == KT - 1))
                for k in range(KT):
                    nc.tensor.matmul(k_ps[:], wkk[:, k * H:(k + 1) * H], xs(k),
                                     start=(k == 0), stop=(k == KT - 1))
                for k in range(KT):
                    nc.tensor.matmul(v_ps[:], wv[:, k * H:(k + 1) * H], xs(k),
                                     start=(k == 0), stop=(k == KT - 1))
                qT = wk.tile([H, N], F32, name="qT", bufs=1)
                kT_ = wk.tile([H, N], F32, name="kT", bufs=1)
                vT = wk.tile([H, N], F32, name="vT", bufs=1)
                nc.vector.tensor_copy(qT[:], q_ps[:])
                nc.vector.tensor_copy(kT_[:], k_ps[:])
                nc.vector.tensor_copy(vT[:], v_ps[:])

                oT = wk.tile([H, N], F32, name="oT", bufs=1)
                for b in range(B):
                    ts_ = slice(b * 64, (b + 1) * 64)
                    w_ps = ps_small.tile([64, 64], F32, tag="sm")
                    nc.tensor.matmul(w_ps[:], qT[:, ts_], kT_[:, ts_],
                                     start=True, stop=True)
                    s_sb = sm.tile([64, 64], F32, tag="att_s")
                    nc.vector.scalar_tensor_tensor(s_sb[:], w_ps[:], SCALE,
                                                   maskb[:], op0=ALU.mult,
                                                   op1=ALU.add)
                    mx = sm.tile([64, 1], F32, tag="att_m")
                    nc.vector.reduce_max(mx[:], s_sb[:], axis=AX.X, negate=True)
                    att = sm.tile([64, 64], F32, tag="att_a")
                    ssum = sm.tile([64, 1], F32, tag="att_su")
                    nc.scalar.activation(att[:], s_sb[:], AF.Exp, bias=mx[:],
                                         accum_out=ssum[:])
                    rs = sm.tile([64, 1], F32, tag="att_r")
                    nc.vector.reciprocal(rs[:], ssum[:])
                    nc.vector.tensor_scalar_mul(att[:], att[:], rs[:])
                    at_ps = ps_small.tile([64, 64], F32, tag="sm")
                    nc.tensor.transpose(at_ps[:], att[:], ident[:64, :64])
                    attT = sm.tile([64, 64], F32, tag="att_t")
                    nc.vector.tensor_copy(attT[:], at_ps[:])
                    vt_ps = ps_small.tile([64, H], F32, tag="sm")
                    nc.tensor.transpose(vt_ps[:], vT[:, ts_], ident[:H, :H])
                    vtb = sm.tile([64, H], F32, tag="att_v")
                    nc.vector.tensor_copy(vtb[:], vt_ps[:])
                    o_ps = ps_small.tile([H, 64], F32, tag="sm")
                    nc.tensor.matmul(o_ps[:], vtb[:], attT[:],
                                     start=True, stop=True)
                    nc.vector.tensor_copy(oT[:, ts_], o_ps[:])

                # proj partials -> ar_in
                for m in range(KT):
                    y_ps = ps_acc.tile([128, N], F32, tag="acc")
                    nc.tensor.matmul(y_ps[:], wp[:, m * 128:(m + 1) * 128],
                                     oT[:], start=True, stop=True)
                    yc = sm.tile([128, N], F32, tag="ycp", bufs=2)
                    nc.vector.tensor_copy(yc[:], y_ps[:])
                    nc.sync.dma_start(ar_ins[2 * l][m * 128:(m + 1) * 128, :],
                                      yc[:])
                nc.gpsimd.collective_compute(
                    "AllReduce", ALU.add,
                    replica_groups=[list(range(N_CORES))],
                    ins=[ar_ins[2 * l][:]], outs=[ar_outs[2 * l][:]])
                yat = wk.tile([128, KT * N], F32, name="yat", bufs=1)
                for k in range(KT):
                    nc.sync.dma_start(yat[:, k * N:(k + 1) * N],
                                      ar_outs[2 * l][k * 128:(k + 1) * 128, :])

                # residual + bproj + ln1  (t = (y + bproj) + x)
                def get_t1(k, yat=yat, bpj=bpj):
                    u = sm.tile([128, N], F32, tag="res_u", bufs=2)
                    nc.vector.scalar_tensor_tensor(
                        u[:], yat[:, k * N:(k + 1) * N], bpj[:, k:k + 1],
                        xs(k), op0=ALU.add, op1=ALU.add)
                    return u[:]

                layernorm(get_t1, l1w, l1b)

                # ---- gate + top2 comb ----
                combT = sm.tile([E, N], F32, tag="combT", bufs=1)
                for tt in range(4):
                    g_ps = ps_small.tile([128, E], F32, tag="sm")
                    for k in range(KT):
                        nc.tensor.matmul(
                            g_ps[:],
                            x_sb[:, k * N + tt * 128:k * N + (tt + 1) * 128],
                            gw[:, k * E:(k + 1) * E],
                            start=(k == 0), stop=False)
                    nc.tensor.matmul(g_ps[:], ones_row[:], gb[:],
                                     start=False, stop=True)
                    gl = sm.tile([128, E], F32, tag="g_l")
                    mx = sm.tile([128, 1], F32, tag="g_m")
                    nc.vector.reduce_max(mx[:], g_ps[:], axis=AX.X, negate=True)
                    pr = sm.tile([128, E], F32, tag="g_p")
                    ssum = sm.tile([128, 1], F32, tag="g_s")
                    nc.scalar.activation(pr[:], g_ps[:], AF.Exp, bias=mx[:],
                                         accum_out=ssum[:])
                    rs = sm.tile([128, 1], F32, tag="g_r")
                    nc.vector.reciprocal(rs[:], ssum[:])
                    nc.vector.tensor_scalar_mul(pr[:], pr[:], rs[:])
                    top8 = sm.tile([128, 8], F32, tag="g_t8")
                    nc.vector.max(out=top8[:], in_=pr[:])
                    msk = sm.tile([128, E], F32, tag="g_msk")
                    nc.vector.tensor_scalar(msk[:], pr[:], top8[:, 1:2], None,
                                            op0=ALU.is_ge)
                    cw = sm.tile([128, E], F32, tag="g_cw")
                    nc.vector.tensor_mul(cw[:], pr[:], msk[:])
                    den = sm.tile([128, 1], F32, tag="g_den")
                    nc.vector.tensor_add(den[:], top8[:, 0:1], top8[:, 1:2])
                    dr = sm.tile([128, 1], F32, tag="g_dr")
                    nc.vector.reciprocal(dr[:], den[:])
                    nc.vector.tensor_scalar_mul(cw[:], cw[:], dr[:])
                    ct_ps = ps_small.tile([E, 128], F32, tag="sm")
                    nc.tensor.transpose(ct_ps[:], cw[:], ident[:])
                    nc.vector.tensor_copy(combT[:, tt * 128:(tt + 1) * 128],
                                          ct_ps[:])
                # this core's expert row -> broadcast [128, N]
                crow_ps = ps_small.tile([1, N], F32, tag="sm")
                nc.tensor.matmul(crow_ps[:], combsel[:], combT[:],
                                 start=True, stop=True)
                crow = sm.tile([1, N], F32, tag="crow", bufs=1)
                nc.vector.tensor_copy(crow[:], crow_ps[:])
                cbc_ps = ps_bc.tile([128, N], F32, tag="bc")
                nc.tensor.matmul(cbc_ps[:], ones_row[:], crow[:],
                                 start=True, stop=True)
                cbc = wk.tile([128, N], F32, name="cbc", bufs=1)
                nc.vector.tensor_copy(cbc[:], cbc_ps[:])

                # ---- MoE expert (dense over all tokens) ----
                hT = hpool.tile([128, MT * N], F32, tag="hT")
                for m in range(MT):
                    w1m = w1pool.tile([128, KT * 128], F32, tag="w1")
                    nc.sync.dma_start(
                        w1m[:].rearrange("p (k f) -> p k f", k=KT),
                        d_w1T[l, m].rearrange("k p f -> p k f"))
                    h_ps = ps_acc.tile([128, N], F32, tag="acc")
                    for k in range(KT):
                        nc.tensor.matmul(
                            h_ps[:], w1m[:, k * 128:(k + 1) * 128],
                            xs(k), start=(k == 0), stop=(k == KT - 1))
                    nc.scalar.activation(hT[:, m * N:(m + 1) * N], h_ps[:],
                                         AF.Gelu, bias=b1t[:, m:m + 1])
                for m in range(KT):
                    w2m = w2pool.tile([128, MT * 128], F32, tag="w2")
                    nc.sync.dma_start(
                        w2m[:].rearrange("p (j f) -> p j f", j=MT),
                        d_w2T[l, m].rearrange("j p f -> p j f"))
                    ye_ps = ps_acc.tile([128, N], F32, tag="acc")
                    for j in range(MT):
                        nc.tensor.matmul(ye_ps[:], w2m[:, j * 128:(j + 1) * 128],
                                         hT[:, j * N:(j + 1) * N],
                                         start=(j == 0), stop=(j == MT - 1))
                    ysc = sm.tile([128, N], F32, tag="ycp", bufs=2)
                    nc.vector.tensor_mul(ysc[:], ye_ps[:], cbc[:])
                    nc.sync.dma_start(ar_ins[2 * l + 1][m * 128:(m + 1) * 128, :],
                                      ysc[:])
                nc.gpsimd.collective_compute(
                    "AllReduce", ALU.add,
                    replica_groups=[list(range(N_CORES))],
                    ins=[ar_ins[2 * l + 1][:]], outs=[ar_outs[2 * l + 1][:]])
                ymoe = wk.tile([128, KT * N], F32, name="yat", bufs=1)
                for k in range(KT):
                    nc.sync.dma_start(ymoe[:, k * N:(k + 1) * N],
                                      ar_outs[2 * l + 1][k * 128:(k + 1) * 128, :])

                # b2 contribution: sum_e comb[t,e]*b2[e,:] = b2.T @ combT
                b2c_list = []
                for k in range(KT):
                    b2c_ps = ps_bc.tile([128, N], F32, tag="bc")
                    nc.tensor.matmul(b2c_ps[:], b2t[:, k * 128:(k + 1) * 128],
                                     combT[:], start=True, stop=True)
                    b2c_list.append(b2c_ps)

                def get_t2(k, ymoe=ymoe, b2c_list=b2c_list):
                    u = sm.tile([128, N], F32, tag="res_u", bufs=2)
                    nc.vector.tensor_add(u[:], ymoe[:, k * N:(k + 1) * N],
                                         b2c_list[k][:])
                    nc.vector.tensor_add(u[:], u[:], xs(k))
                    return u[:]

                layernorm(get_t2, l2w, l2b)

            # ---- final ln + lm head ----
            def get_tf(k):
                return xs(k)

            layernorm(get_tf, lnfw, lnfb)
            lg_ps = ps_acc.tile([V, N], F32, tag="acc")
            for k in range(KT):
                nc.tensor.matmul(lg_ps[:], lmT[:, k * V:(k + 1) * V], xs(k),
                                 start=(k == 0), stop=(k == KT - 1))
            lg = sm.tile([V, N], F32, tag="lgout", bufs=1)
            nc.scalar.activation(lg[:], lg_ps[:], AF.Identity, bias=lmb[:])
            nc.sync.dma_start(d_out[:], lg[:])

    return nc


def _prep(inputs):
    """Build per-core input maps from the full input dict."""
    f = lambda a: np.ascontiguousarray(np.asarray(a), dtype=np.float32)
    idx = np.asarray(inputs["idx"]).reshape(1, N)
    wq, wkk, wv = f(inputs["wq"]), f(inputs["wk"]), f(inputs["wv"])
    wproj, bproj = f(inputs["wproj"]), f(inputs["bproj"])
    gate_w, gate_b = f(inputs["gate_w"]), f(inputs["gate_b"])
    w1, b1 = f(inputs["w1"]), f(inputs["b1"])
    w2, b2 = f(inputs["w2"]), f(inputs["b2"])

    base = {
        "idx": np.ascontiguousarray(idx.astype(np.int32)),
        "iota99": np.arange(V, dtype=np.float32).reshape(V, 1),
        "ident128": np.eye(128, dtype=np.float32),
        "maskb": np.where(np.tril(np.ones((64, 64), bool)), 0.0,
                          -1e30).astype(np.float32),
        "ones_col": np.ones((128, 1), np.float32),
        "ones_row": np.ones((1, 128), np.float32),
        "tok_emb": f(inputs["tok_emb"]),
        "posT": np.ascontiguousarray(
            np.tile(f(inputs["pos_emb"]).T, (1, B))),
        "gwT": np.ascontiguousarray(
            gate_w.transpose(0, 2, 1).reshape(L, KT, 128, E)),
        "gb": gate_b.reshape(L, 1, E),
        "b2all": b2,
        "ln1w": f(inputs["ln1_w"]).reshape(L, KT, 128),
        "ln1b": f(inputs["ln1_b"]).reshape(L, KT, 128),
        "ln2w": f(inputs["ln2_w"]).reshape(L, KT, 128),
        "ln2b": f(inputs["ln2_b"]).reshape(L, KT, 128),
        "lnfw": f(inputs["lnf_w"]).reshape(KT, 128),
        "lnfb": f(inputs["lnf_b"]).reshape(KT, 128),
        "lmT": np.ascontiguousarray(f(inputs["lm_w"]).T.reshape(KT, 128, V)),
        "lmb": f(inputs["lm_b"]).reshape(V, 1),
        "bproj": bproj.reshape(L, KT, 128),
    }
    in_maps = []
    for c in range(N_CORES):
        m = dict(base)
        m["wqT"] = np.ascontiguousarray(
            wq[:, c].transpose(0, 2, 1).reshape(L, KT, 128, H))
        m["wkT"] = np.ascontiguousarray(
            wkk[:, c].transpose(0, 2, 1).reshape(L, KT, 128, H))
        m["wvT"] = np.ascontiguousarray(
            wv[:, c].transpose(0, 2, 1).reshape(L, KT, 128, H))
        m["wpT"] = np.ascontiguousarray(
            wproj[:, :, c * H:(c + 1) * H].transpose(0, 2, 1))
        w1tc = w1[:, c].transpose(0, 2, 1)  # [L, 768, 3072]
        m["w1T"] = np.ascontiguousarray(
            w1tc.reshape(L, KT, 128, MT, 128).transpose(0, 3, 1, 2, 4))
        m["b1"] = np.ascontiguousarray(b1[:, c].reshape(L, MT, 128))
        w2tc = w2[:, c].transpose(0, 2, 1)  # [L, 3072, 768]
        m["w2T"] = np.ascontiguousarray(
            w2tc.reshape(L, MT, 128, KT, 128).transpose(0, 3, 1, 2, 4))
        sel = np.zeros((E, 1), np.float32)
        sel[c, 0] = 1.0
        m["combsel"] = sel
        in_maps.append(m)
    return in_maps


def kernel(**inputs) -> np.ndarray:
    if "nc" not in _CACHED:
        _CACHED["nc"] = build()
    nc = _CACHED["nc"]
    in_maps = _prep(inputs)
    res = run_bass_kernel_spmd(nc, in_maps, list(range(N_CORES)))
    lt = res.results[0]["logitsT"]  # [V, N]
    return np.ascontiguousarray(lt.T.reshape(B, T, V).astype(np.float32))


if __name__ == "__main__":
    import jax

    jax.config.update("jax_platforms", "cpu")
    import reference as ref

    inp = ref.setup_inputs()
    want = np.asarray(ref.reference(**inp))
    import jax as _j
    _j.config.update("jax_platforms", "axon")
    got = kernel(**{k: np.asarray(v) for k, v in inp.items()})
    err = np.abs(got - want).max()
    rel = err / np.abs(want).max()
    l2 = np.linalg.norm(got - want) / np.linalg.norm(want)
    print(f"absmax {err:.3e}  absmax-rel {rel:.3e}  l2-rel {l2:.3e}")



# revision 11
# speedup vs baseline: 1.2210x; 1.2210x over previous
"""Trainium2 Bass kernel for nn_CustomTransformer_58445914964311.

12-layer MoE transformer (768 embd, 8 heads, 8 experts top-2, B=8 x T=64
tokens), distributed over 8 NeuronCores:
  - attention sharded by head (core c computes head c for all 512 tokens),
  - MoE sharded by expert (core c computes expert c densely for all tokens,
    weighted by its combine weight),
  - per-layer AllReduce combines the per-head attention partials and the
    per-expert MoE partials.

Optimizations over the plain-fp32 version:
  - heavy GEMMs (qkv, proj, w1, w2) run as a 3-matmul bf16 split
    (Ah@Wh + Ah@Wl + Al@Wh, ~16 effective mantissa bits, fp32 PSUM
    accumulate): 3 cyc/row instead of fp32's 4, half the weight DMA, and
    bf16 gets fast-weight-load. Verified on CPU to reproduce all 6144
    top-2 gate decisions for this input (final rel err ~7e-6).
  - attention is processed in two 256-token chunks with per-chunk
    AllReduces hidden under the other chunk's compute.
  - the MoE output AllReduce is split into 6 per-feature-tile AllReduces
    pipelined under the remaining w2 matmuls; LN2 consumes each tile as
    it lands.
  - routing-critical math (gate, layernorms, attention inner, residuals)
    stays in plain fp32.

Self-contained: hardcodes all shapes; host side only reshapes/transposes/
splits the incoming fp32 weights.
"""

import numpy as np
import ml_dtypes

import concourse.bass as bass
import concourse.mybir as mybir
import concourse.tile as tile
from concourse.bass_utils import run_bass_kernel_spmd

import os
import sys

# ---------------------------------------------------------------------------
# Compatibility patches (inlined): the walrus build here rejects instructions
# carrying more than one semaphore wait ("Too many sync wait commands").
# 1) split the Tile kernel-tail drain's waits onto separate sync nops;
# 2) post-process the serialized BIR, peeling extra waits onto injected
#    EventSemaphore instructions;
# 3) recreate the missing antenv.axon_hooks registry so trace=True works.
# ---------------------------------------------------------------------------
import orjson as _orjson
from concourse.vector_clock import ScopedClock as _ScopedClock

_COMPAT_DONE = False


def _patched_drain_and_barrier(self, tick_clock, wait_clock):
    nc = self.nc
    collector = nc.sync.nop()
    wait_clock.add_sem_waits(
        collector.ins, _ScopedClock({None: tick_clock.global_clock})
    )
    si = collector.ins.sync_info
    waits = list(si.on_wait or []) if si is not None else []
    if len(waits) > 1:
        si.on_wait = waits[:1]
        for w in waits[1:]:
            extra = nc.sync.nop()
            esi = extra.ins.sync_info
            if esi is None:
                extra.ins.sync_info = mybir.SyncInfo(on_wait=[w], on_update=[])
            else:
                esi.on_wait = [w]
    nc.sync.drain()
    nc.all_engine_barrier()
    popped = nc._tile_sem_poison_stack.pop()
    assert popped is self._sem_poison
    nc.clear_and_free_semaphores(list(self.sems.allocated().values()))
    nc.all_engine_barrier()


def _split_multi_waits(mod, max_waits=1):
    ctr = 0
    for fn in mod.get("functions", []):
        for blk in fn.get("blocks", []):
            insts = blk.get("instructions", [])
            if not any(
                len((i.get("sync_info") or {}).get("on_wait") or []) > max_waits
                for i in insts
            ):
                continue
            new_insts = []
            for inst in insts:
                si = inst.get("sync_info")
                waits = (si.get("on_wait") or []) if si else []
                if len(waits) > max_waits:
                    for w in waits[max_waits:]:
                        ctr += 1
                        new_insts.append({
                            "debug": inst.get("debug", 0),
                            "engine": inst["engine"],
                            "ins": [], "outs": [],
                            "name": f"{inst['name']}-wsp{ctr}",
                            "opcode": "EventSemaphore",
                            "sync_info": {"on_update": [], "on_wait": [w]},
                        })
                    si["on_wait"] = waits[:max_waits]
                new_insts.append(inst)
            blk["instructions"] = new_insts
    return mod


_orig_to_json_bytes = bass.Bass.to_json_bytes


def _patched_to_json_bytes(self):
    return _orjson.dumps(_split_multi_waits(_orjson.loads(_orig_to_json_bytes(self))))


def _install_ntff_hook_shim():
    import types
    if "antenv.axon_hooks" in sys.modules:
        return
    try:
        import antenv  # noqa: F401
    except ImportError:
        return
    mod = types.ModuleType("antenv.axon_hooks")
    _state = {"hook": None}
    mod.set_axon_ntff_profile_hook = lambda hook: _state.__setitem__("hook", hook)
    mod.get_axon_ntff_profile_hook = lambda: _state["hook"]
    sys.modules["antenv.axon_hooks"] = mod
    sys.modules["antenv"].axon_hooks = mod
    try:
        from trn_agent_boot.trn_boot import _ntff_profile_via_ctypes
        hook = _ntff_profile_via_ctypes("/opt/axon/libaxon_pjrt.so")
        if hook is not None:
            mod.set_axon_ntff_profile_hook(hook)
    except Exception:
        pass


def _install_compat():
    global _COMPAT_DONE
    if _COMPAT_DONE:
        return
    tile.TileContext._drain_and_barrier = _patched_drain_and_barrier
    bass.Bass.to_json_bytes = _patched_to_json_bytes
    _install_ntff_hook_shim()
    _COMPAT_DONE = True


_install_compat()

F32 = mybir.dt.float32
BF16 = mybir.dt.bfloat16
I32 = mybir.dt.int32
AF = mybir.ActivationFunctionType
ALU = mybir.AluOpType
AX = mybir.AxisListType

N_CORES = 8
L = 12
D = 768
H = 96          # head dim
NH = 8
E = 8           # experts
DFF = 3072
B, T = 8, 64
N = B * T       # 512 tokens
NC = 256        # token chunk (2 chunks)
V = 99
KT = D // 128   # 6 feature tiles
MT = DFF // 128  # 24 dff tiles
EPS = 1e-5
SCALE = H ** -0.5

_CACHED = {}


def build():
    nc = bass.Bass(num_devices=N_CORES)

    # ---- inputs (per-core data, same names) ----
    d_idx = nc.dram_tensor("idx", [1, N], I32, kind="ExternalInput")
    d_iota = nc.dram_tensor("iota99", [V, 1], F32, kind="ExternalInput")
    d_ident = nc.dram_tensor("ident128", [128, 128], F32, kind="ExternalInput")
    d_mask = nc.dram_tensor("maskb", [64, 64], F32, kind="ExternalInput")
    d_ones_col = nc.dram_tensor("ones_col", [128, 1], F32, kind="ExternalInput")
    d_ones_row = nc.dram_tensor("ones_row", [1, 128], F32, kind="ExternalInput")
    d_tok = nc.dram_tensor("tok_emb", [V, D], F32, kind="ExternalInput")
    d_posT = nc.dram_tensor("posT", [D, N], F32, kind="ExternalInput")
    # attention weights, bf16 hi/lo splits
    d_wqh = nc.dram_tensor("wqh", [L, KT, 128, H], BF16, kind="ExternalInput")
    d_wql = nc.dram_tensor("wql", [L, KT, 128, H], BF16, kind="ExternalInput")
    d_wkh = nc.dram_tensor("wkh", [L, KT, 128, H], BF16, kind="ExternalInput")
    d_wkl = nc.dram_tensor("wkl", [L, KT, 128, H], BF16, kind="ExternalInput")
    d_wvh = nc.dram_tensor("wvh", [L, KT, 128, H], BF16, kind="ExternalInput")
    d_wvl = nc.dram_tensor("wvl", [L, KT, 128, H], BF16, kind="ExternalInput")
    d_wph = nc.dram_tensor("wph", [L, H, D], BF16, kind="ExternalInput")
    d_wpl = nc.dram_tensor("wpl", [L, H, D], BF16, kind="ExternalInput")
    d_bproj = nc.dram_tensor("bproj", [L, KT, 128], F32, kind="ExternalInput")
    d_gwT = nc.dram_tensor("gwT", [L, KT, 128, E], F32, kind="ExternalInput")
    d_gb = nc.dram_tensor("gb", [L, 1, E], F32, kind="ExternalInput")
    # MoE weights, bf16 hi/lo splits
    d_w1h = nc.dram_tensor("w1h", [L, MT, KT, 128, 128], BF16, kind="ExternalInput")
    d_w1l = nc.dram_tensor("w1l", [L, MT, KT, 128, 128], BF16, kind="ExternalInput")
    d_b1 = nc.dram_tensor("b1", [L, MT, 128], F32, kind="ExternalInput")
    d_w2h = nc.dram_tensor("w2h", [L, KT, MT, 128, 128], BF16, kind="ExternalInput")
    d_w2l = nc.dram_tensor("w2l", [L, KT, MT, 128, 128], BF16, kind="ExternalInput")
    d_b2 = nc.dram_tensor("b2all", [L, E, D], F32, kind="ExternalInput")
    d_combsel = nc.dram_tensor("combsel", [E, 1], F32, kind="ExternalInput")
    d_ln1w = nc.dram_tensor("ln1w", [L, KT, 128], F32, kind="ExternalInput")
    d_ln1b = nc.dram_tensor("ln1b", [L, KT, 128], F32, kind="ExternalInput")
    d_ln2w = nc.dram_tensor("ln2w", [L, KT, 128], F32, kind="ExternalInput")
    d_ln2b = nc.dram_tensor("ln2b", [L, KT, 128], F32, kind="ExternalInput")
    d_lnfw = nc.dram_tensor("lnfw", [KT, 128], F32, kind="ExternalInput")
    d_lnfb = nc.dram_tensor("lnfb", [KT, 128], F32, kind="ExternalInput")
    d_lmT = nc.dram_tensor("lmT", [KT, 128, V], F32, kind="ExternalInput")
    d_lmb = nc.dram_tensor("lmb", [V, 1], F32, kind="ExternalInput")
    d_out = nc.dram_tensor("logitsT", [V, N], F32, kind="ExternalOutput")

    with tile.TileContext(nc) as tc:
        with (
            tc.tile_pool(name="const", bufs=1) as cpool,
            tc.tile_pool(name="x", bufs=1) as xpool,
            tc.tile_pool(name="attw", bufs=2) as awpool,
            tc.tile_pool(name="w1", bufs=2) as w1pool,
            tc.tile_pool(name="w2", bufs=2) as w2pool,
            tc.tile_pool(name="h", bufs=1) as hpool,
            tc.tile_pool(name="work", bufs=2) as wk,
            tc.tile_pool(name="small", bufs=3) as sm,
            tc.tile_pool(name="ps_big", bufs=2, space="PSUM") as ps_big,
            tc.tile_pool(name="ps_att", bufs=2, space="PSUM") as ps_att,
            tc.tile_pool(name="ps_small", bufs=2, space="PSUM") as ps_small,
            tc.tile_pool(name="ps_bc", bufs=2, space="PSUM") as ps_bc,
            tc.tile_pool(name="dram", bufs=1, space="DRAM") as dpool,
        ):
            # ---- constants resident ----
            ident = cpool.tile([128, 128], F32, name="ident")
            nc.sync.dma_start(ident[:], d_ident[:])
            maskb = cpool.tile([64, 64], F32, name="maskb")
            nc.sync.dma_start(maskb[:], d_mask[:])
            iota99 = cpool.tile([V, 1], F32, name="iota99")
            nc.sync.dma_start(iota99[:], d_iota[:])
            ones_col = cpool.tile([128, 1], F32, name="ones_col")
            nc.sync.dma_start(ones_col[:], d_ones_col[:])
            ones_row = cpool.tile([1, 128], F32, name="ones_row")
            nc.sync.dma_start(ones_row[:], d_ones_row[:])
            combsel = cpool.tile([E, 1], F32, name="combsel")
            nc.sync.dma_start(combsel[:], d_combsel[:])
            tok = cpool.tile([V, D], F32, name="tok")
            nc.sync.dma_start(tok[:], d_tok[:])
            lmT = cpool.tile([128, KT * V], F32, name="lmT")
            for k in range(KT):
                nc.sync.dma_start(lmT[:, k * V:(k + 1) * V], d_lmT[k])
            lmb = cpool.tile([V, 1], F32, name="lmb")
            nc.sync.dma_start(lmb[:], d_lmb[:])
            lnfw = cpool.tile([128, KT], F32, name="lnfw")
            nc.sync.dma_start(lnfw[:], d_lnfw.rearrange("a p -> p a"))
            lnfb = cpool.tile([128, KT], F32, name="lnfb")
            nc.sync.dma_start(lnfb[:], d_lnfb.rearrange("a p -> p a"))

            # AR bounce tensors: attention per (layer, chunk); MoE per (layer, k)
            ar_att_in = [[dpool.tile([D, NC], F32, name=f"aati{l}_{c}")
                          for c in range(2)] for l in range(L)]
            ar_att_out = [[dpool.tile([D, NC], F32, name=f"aato{l}_{c}",
                                      addr_space="Shared")
                           for c in range(2)] for l in range(L)]
            ar_moe_in = [[dpool.tile([128, N], F32, name=f"amoi{l}_{k}")
                          for k in range(KT)] for l in range(L)]
            ar_moe_out = [[dpool.tile([128, N], F32, name=f"amoo{l}_{k}",
                                      addr_space="Shared")
                           for k in range(KT)] for l in range(L)]

            # ---- x state: fp32 + bf16 hi/lo splits ----
            x_sb = xpool.tile([128, KT * N], F32, name="x_sb")
            xh_sb = xpool.tile([128, KT * N], BF16, name="xh_sb")
            xl_sb = xpool.tile([128, KT * N], BF16, name="xl_sb")

            def xs(k, cols=slice(0, N)):
                return x_sb[:, k * N + cols.start: k * N + cols.stop]

            def xhs(k, cols=slice(0, N)):
                return xh_sb[:, k * N + cols.start: k * N + cols.stop]

            def xls(k, cols=slice(0, N)):
                return xl_sb[:, k * N + cols.start: k * N + cols.stop]

            def split_x(k, cols):
                """bf16 hi/lo split of x_sb slice into xh/xl."""
                nc.vector.tensor_copy(xhs(k, cols), xs(k, cols))
                nc.vector.tensor_sub(xls(k, cols), xs(k, cols), xhs(k, cols))

            # ---- embedding ----
            idx_i = sm.tile([1, N], I32, name="idx_i", bufs=1)
            nc.sync.dma_start(idx_i[:], d_idx[:])
            idx_f = sm.tile([1, N], F32, name="idx_f", bufs=1)
            nc.vector.tensor_copy(idx_f[:], idx_i[:])
            idxbc = ps_bc.tile([V, N], F32, tag="bc")
            nc.tensor.matmul(idxbc[:], ones_row[:, :V], idx_f[:],
                             start=True, stop=True)
            onehot = wk.tile([V, N], F32, name="onehot", bufs=1)
            nc.vector.tensor_scalar(onehot[:], idxbc[:], iota99[:], None,
                                    op0=ALU.is_equal)
            for k in range(KT):
                posk = wk.tile([128, N], F32, name="posk", bufs=2)
                nc.sync.dma_start(posk[:], d_posT[k * 128:(k + 1) * 128, :])
                e_ps = ps_big.tile([128, N], F32, tag="acc")
                nc.tensor.matmul(e_ps[:], tok[:, k * 128:(k + 1) * 128],
                                 onehot[:], start=True, stop=True)
                nc.vector.tensor_add(xs(k), e_ps[:], posk[:])
                split_x(k, slice(0, N))

            def layernorm_cols(cols, w_ap, b_ap, presummed=False,
                               s_ps=None, q_ps=None):
                """Normalize x_sb[:, cols] in place (pre-norm values already
                there) and refresh the bf16 splits. If presummed, s_ps/q_ps
                already hold sum and sumsq (PE-accumulated)."""
                W = cols.stop - cols.start
                if not presummed:
                    s_ps = ps_small.tile([1, W], F32, tag="sm")
                    q_ps = ps_small.tile([1, W], F32, tag="sm")
                    for k in range(KT):
                        tk = xs(k, cols)
                        sq = sm.tile([128, W], F32, tag="lnsq", bufs=2)
                        nc.scalar.activation(sq[:], tk, AF.Square)
                        nc.tensor.matmul(s_ps[:], ones_col[:], tk,
                                         start=(k == 0), stop=(k == KT - 1))
                        nc.tensor.matmul(q_ps[:], ones_col[:], sq[:],
                                         start=(k == 0), stop=(k == KT - 1))
                # [1,W] scratch at base partition 0 (AP base-partition rule)
                var_t = sm.tile([1, N], F32, tag="lnvar", bufs=2)
                rstd_t = sm.tile([1, N], F32, tag="lnrstd", bufs=2)
                nmu_t = sm.tile([1, N], F32, tag="lnnmu", bufs=2)
                var = var_t[:, :W]
                rstd = rstd_t[:, :W]
                nmu = nmu_t[:, :W]
                nc.vector.tensor_scalar_mul(nmu, s_ps[:], -1.0 / D)
                nc.vector.tensor_mul(var, nmu, nmu)  # mu^2
                nc.vector.scalar_tensor_tensor(var, q_ps[:], 1.0 / D, var,
                                               op0=ALU.mult, op1=ALU.subtract)
                nc.vector.tensor_scalar_add(var, var, EPS)
                nc.scalar.activation(var, var, AF.Sqrt)
                nc.vector.reciprocal(rstd, var)
                nmu_bc = ps_bc.tile([128, W], F32, tag="bc")
                nc.tensor.matmul(nmu_bc[:], ones_row[:], nmu,
                                 start=True, stop=True)
                rstd_bc = ps_bc.tile([128, W], F32, tag="bc")
                nc.tensor.matmul(rstd_bc[:], ones_row[:], rstd,
                                 start=True, stop=True)
                for k in range(KT):
                    u = sm.tile([128, W], F32, tag="lnu", bufs=2)
                    nc.vector.tensor_add(u[:], xs(k, cols), nmu_bc[:])
                    nc.vector.tensor_mul(u[:], u[:], rstd_bc[:])
                    nc.vector.tensor_scalar(xs(k, cols), u[:], w_ap[:, k:k + 1],
                                            b_ap[:, k:k + 1],
                                            op0=ALU.mult, op1=ALU.add)
                    split_x(k, cols)

            for l in range(L):
                # ---- stage layer weights (attw pool, bufs=2 -> prefetch) ----
                wqh = awpool.tile([128, KT * H], BF16, tag="wqh")
                wql = awpool.tile([128, KT * H], BF16, tag="wql")
                wkh = awpool.tile([128, KT * H], BF16, tag="wkh")
                wkl = awpool.tile([128, KT * H], BF16, tag="wkl")
                wvh = awpool.tile([128, KT * H], BF16, tag="wvh")
                wvl = awpool.tile([128, KT * H], BF16, tag="wvl")
                for k in range(KT):
                    sl = slice(k * H, (k + 1) * H)
                    nc.sync.dma_start(wqh[:, sl], d_wqh[l, k])
                    nc.sync.dma_start(wql[:, sl], d_wql[l, k])
                    nc.sync.dma_start(wkh[:, sl], d_wkh[l, k])
                    nc.sync.dma_start(wkl[:, sl], d_wkl[l, k])
                    nc.sync.dma_start(wvh[:, sl], d_wvh[l, k])
                    nc.sync.dma_start(wvl[:, sl], d_wvl[l, k])
                wph = awpool.tile([H, D], BF16, tag="wph")
                nc.sync.dma_start(wph[:], d_wph[l])
                wpl = awpool.tile([H, D], BF16, tag="wpl")
                nc.sync.dma_start(wpl[:], d_wpl[l])
                bpj = awpool.tile([128, KT], F32, tag="bpj")
                nc.sync.dma_start(bpj[:], d_bproj[l].rearrange("a p -> p a"))
                gw = awpool.tile([128, KT * E], F32, tag="gw")
                for k in range(KT):
                    nc.sync.dma_start(gw[:, k * E:(k + 1) * E], d_gwT[l, k])
                gb = awpool.tile([1, E], F32, tag="gb")
                nc.sync.dma_start(gb[:], d_gb[l])
                l1w = awpool.tile([128, KT], F32, tag="l1w")
                nc.sync.dma_start(l1w[:], d_ln1w[l].rearrange("a p -> p a"))
                l1b = awpool.tile([128, KT], F32, tag="l1b")
                nc.sync.dma_start(l1b[:], d_ln1b[l].rearrange("a p -> p a"))
                l2w = awpool.tile([128, KT], F32, tag="l2w")
                nc.sync.dma_start(l2w[:], d_ln2w[l].rearrange("a p -> p a"))
                l2b = awpool.tile([128, KT], F32, tag="l2b")
                nc.sync.dma_start(l2b[:], d_ln2b[l].rearrange("a p -> p a"))
                b1t = awpool.tile([128, MT], F32, tag="b1t")
                nc.sync.dma_start(b1t[:], d_b1[l].rearrange("a p -> p a"))
                b2t = awpool.tile([E, D], F32, tag="b2t")
                nc.sync.dma_start(b2t[:], d_b2[l])

                # ---- attention (this core's head), 2 token chunks ----
                qT = wk.tile([H, N], F32, name="qT", bufs=1)
                kT_ = wk.tile([H, N], F32, name="kT", bufs=1)
                vT = wk.tile([H, N], F32, name="vT", bufs=1)
                oT = wk.tile([H, N], F32, name="oT", bufs=1)
                oh = wk.tile([H, N], BF16, name="oh", bufs=1)
                ol = wk.tile([H, N], BF16, name="ol", bufs=1)

                def split3(psum, whi, wlo, rhs_h, rhs_l, kslices, first, last):
                    """Accumulate 3-split matmuls into psum over kslices."""
                    nk = len(kslices)
                    for i, (wsl, rh, rl) in enumerate(kslices):
                        nc.tensor.matmul(psum, whi[:, wsl], rh,
                                         start=(first and i == 0), stop=False)
                        nc.tensor.matmul(psum, whi[:, wsl], rl,
                                         start=False, stop=False)
                        nc.tensor.matmul(psum, wlo[:, wsl], rh,
                                         start=False,
                                         stop=(last and i == nk - 1))

                for c in range(2):
                    cols = slice(c * NC, (c + 1) * NC)
                    ks = [(slice(k * H, (k + 1) * H), xhs(k, cols), xls(k, cols))
                          for k in range(KT)]
                    q_ps = ps_att.tile([H, NC], F32, tag="att")
                    split3(q_ps[:], wqh, wql, None, None, ks, True, True)
                    nc.vector.tensor_copy(qT[:, cols], q_ps[:])
                    k_ps = ps_att.tile([H, NC], F32, tag="att")
                    split3(k_ps[:], wkh, wkl, None, None, ks, True, True)
                    nc.vector.tensor_copy(kT_[:, cols], k_ps[:])
                    v_ps = ps_att.tile([H, NC], F32, tag="att")
                    split3(v_ps[:], wvh, wvl, None, None, ks, True, True)
                    nc.vector.tensor_copy(vT[:, cols], v_ps[:])

                    # inner attention, fp32, per batch row
                    for b in range(c * 4, c * 4 + 4):
                        ts_ = slice(b * 64, (b + 1) * 64)
                        w_ps = ps_small.tile([64, 64], F32, tag="sm")
                        nc.tensor.matmul(w_ps[:], qT[:, ts_], kT_[:, ts_],
                                         start=True, stop=True)
                        s_sb = sm.tile([64, 64], F32, tag="att_s")
                        nc.vector.scalar_tensor_tensor(s_sb[:], w_ps[:], SCALE,
                                                       maskb[:], op0=ALU.mult,
                                                       op1=ALU.add)
                        mx = sm.tile([64, 1], F32, tag="att_m")
                        nc.vector.reduce_max(mx[:], s_sb[:], axis=AX.X,
                                             negate=True)
                        att = sm.tile([64, 64], F32, tag="att_a")
                        ssum = sm.tile([64, 1], F32, tag="att_su")
                        nc.scalar.activation(att[:], s_sb[:], AF.Exp, bias=mx[:],
                                             accum_out=ssum[:])
                        rs = sm.tile([64, 1], F32, tag="att_r")
                        nc.vector.reciprocal(rs[:], ssum[:])
                        nc.vector.tensor_scalar_mul(att[:], att[:], rs[:])
                        at_ps = ps_small.tile([64, 64], F32, tag="sm")
                        nc.tensor.transpose(at_ps[:], att[:], ident[:64, :64])
                        attT = sm.tile([64, 64], F32, tag="att_t")
                        nc.vector.tensor_copy(attT[:], at_ps[:])
                        vt_ps = ps_small.tile([64, H], F32, tag="sm")
                        nc.tensor.transpose(vt_ps[:], vT[:, ts_], ident[:H, :H])
                        vtb = sm.tile([64, H], F32, tag="att_v")
                        nc.vector.tensor_copy(vtb[:], vt_ps[:])
                        o_ps = ps_small.tile([H, 64], F32, tag="sm")
                        nc.tensor.matmul(o_ps[:], vtb[:], attT[:],
                                         start=True, stop=True)
                        nc.vector.tensor_copy(oT[:, ts_], o_ps[:])

                    # split o for bf16 proj
                    nc.vector.tensor_copy(oh[:, cols], oT[:, cols])
                    nc.vector.tensor_sub(ol[:, cols], oT[:, cols], oh[:, cols])

                    # proj partials -> ar_att_in[c]
                    for m in range(KT):
                        msl = slice(m * 128, (m + 1) * 128)
                        y_ps = ps_att.tile([128, NC], F32, tag="att")
                        nc.tensor.matmul(y_ps[:], wph[:, msl], oh[:, cols],
                                         start=True, stop=False)
                        nc.tensor.matmul(y_ps[:], wph[:, msl], ol[:, cols],
                                         start=False, stop=False)
                        nc.tensor.matmul(y_ps[:], wpl[:, msl], oh[:, cols],
                                         start=False, stop=True)
                        yc = sm.tile([128, NC], F32, tag="ycp", bufs=2)
                        nc.vector.tensor_copy(yc[:], y_ps[:])
                        nc.sync.dma_start(ar_att_in[l][c][msl, :], yc[:])
                    nc.gpsimd.collective_compute(
                        "AllReduce", ALU.add,
                        replica_groups=[list(range(N_CORES))],
                        ins=[ar_att_in[l][c][:]], outs=[ar_att_out[l][c][:]])

                combT = sm.tile([E, N], F32, tag="combT", bufs=1)
                cbc = wk.tile([128, N], F32, name="cbc", bufs=1)

                for c in range(2):
                    cols = slice(c * NC, (c + 1) * NC)
                    # residual + bproj + ln1 for this chunk (in place on x)
                    yat = wk.tile([128, KT * NC], F32, name="yat", bufs=1)
                    for k in range(KT):
                        nc.sync.dma_start(
                            yat[:, k * NC:(k + 1) * NC],
                            ar_att_out[l][c][k * 128:(k + 1) * 128, :])
                    for k in range(KT):
                        nc.vector.scalar_tensor_tensor(
                            xs(k, cols),
                            yat[:, k * NC:(k + 1) * NC], bpj[:, k:k + 1],
                            xs(k, cols), op0=ALU.add, op1=ALU.add)

                    layernorm_cols(cols, l1w, l1b)

                    # gate + top2 comb for this chunk (2 tt of 128 tokens)
                    for tt in range(2):
                        tcol = slice(c * NC + tt * 128, c * NC + (tt + 1) * 128)
                        g_ps = ps_small.tile([128, E], F32, tag="sm")
                        for k in range(KT):
                            nc.tensor.matmul(
                                g_ps[:], xs(k, tcol),
                                gw[:, k * E:(k + 1) * E],
                                start=(k == 0), stop=False)
                        nc.tensor.matmul(g_ps[:], ones_row[:], gb[:],
                                         start=False, stop=True)
                        mx = sm.tile([128, 1], F32, tag="g_m")
                        nc.vector.reduce_max(mx[:], g_ps[:], axis=AX.X,
                                             negate=True)
                        pr = sm.tile([128, E], F32, tag="g_p")
                        ssum = sm.tile([128, 1], F32, tag="g_s")
                        nc.scalar.activation(pr[:], g_ps[:], AF.Exp, bias=mx[:],
                                             accum_out=ssum[:])
                        rs = sm.tile([128, 1], F32, tag="g_r")
                        nc.vector.reciprocal(rs[:], ssum[:])
                        nc.vector.tensor_scalar_mul(pr[:], pr[:], rs[:])
                        top8 = sm.tile([128, 8], F32, tag="g_t8")
                        nc.vector.max(out=top8[:], in_=pr[:])
                        msk = sm.tile([128, E], F32, tag="g_msk")
                        nc.vector.tensor_scalar(msk[:], pr[:], top8[:, 1:2],
                                                None, op0=ALU.is_ge)
                        cw = sm.tile([128, E], F32, tag="g_cw")
                        nc.vector.tensor_mul(cw[:], pr[:], msk[:])
                        den = sm.tile([128, 1], F32, tag="g_den")
                        nc.vector.tensor_add(den[:], top8[:, 0:1], top8[:, 1:2])
                        dr = sm.tile([128, 1], F32, tag="g_dr")
                        nc.vector.reciprocal(dr[:], den[:])
                        nc.vector.tensor_scalar_mul(cw[:], cw[:], dr[:])
                        ct_ps = ps_small.tile([E, 128], F32, tag="sm")
                        nc.tensor.transpose(ct_ps[:], cw[:], ident[:])
                        nc.vector.tensor_copy(combT[:, tcol], ct_ps[:])
                    # this core's expert row -> broadcast [128, NC]
                    crow_ps = ps_small.tile([1, NC], F32, tag="sm")
                    nc.tensor.matmul(crow_ps[:], combsel[:], combT[:, cols],
                                     start=True, stop=True)
                    crow = sm.tile([1, NC], F32, tag="crow", bufs=2)
                    nc.vector.tensor_copy(crow[:], crow_ps[:])
                    cbc_ps = ps_bc.tile([128, NC], F32, tag="bc")
                    nc.tensor.matmul(cbc_ps[:], ones_row[:], crow[:],
                                     start=True, stop=True)
                    nc.vector.tensor_copy(cbc[:, cols], cbc_ps[:])

                # ---- MoE expert (dense over all tokens), full width ----
                hh = hpool.tile([128, MT * N], BF16, tag="hh")
                hl = hpool.tile([128, MT * N], BF16, tag="hl")
                for m in range(MT):
                    w1m = w1pool.tile([128, 2 * KT * 128], BF16, tag="w1")
                    nc.sync.dma_start(
                        w1m[:, :KT * 128].rearrange("p (k f) -> p k f", k=KT),
                        d_w1h[l, m].rearrange("k p f -> p k f"))
                    nc.sync.dma_start(
                        w1m[:, KT * 128:].rearrange("p (k f) -> p k f", k=KT),
                        d_w1l[l, m].rearrange("k p f -> p k f"))
                    h_ps = ps_big.tile([128, N], F32, tag="acc")
                    for k in range(KT):
                        ksl = slice(k * 128, (k + 1) * 128)
                        ksl2 = slice(KT * 128 + k * 128, KT * 128 + (k + 1) * 128)
                        nc.tensor.matmul(h_ps[:], w1m[:, ksl], xhs(k),
                                         start=(k == 0), stop=False)
                        nc.tensor.matmul(h_ps[:], w1m[:, ksl], xls(k),
                                         start=False, stop=False)
                        nc.tensor.matmul(h_ps[:], w1m[:, ksl2], xhs(k),
                                         start=False, stop=(k == KT - 1))
                    hm = sm.tile([128, N], F32, tag="hm", bufs=2)
                    nc.scalar.activation(hm[:], h_ps[:], AF.Gelu,
                                         bias=b1t[:, m:m + 1])
                    nc.vector.tensor_copy(hh[:, m * N:(m + 1) * N], hm[:])
                    nc.vector.tensor_sub(hl[:, m * N:(m + 1) * N], hm[:],
                                         hh[:, m * N:(m + 1) * N])

                # w2 k-outer; per-k AllReduce pipelined under remaining k's
                for k in range(KT):
                    w2k = w2pool.tile([128, 2 * MT * 128], BF16, tag="w2")
                    nc.sync.dma_start(
                        w2k[:, :MT * 128].rearrange("p (j f) -> p j f", j=MT),
                        d_w2h[l, k].rearrange("j p f -> p j f"))
                    nc.sync.dma_start(
                        w2k[:, MT * 128:].rearrange("p (j f) -> p j f", j=MT),
                        d_w2l[l, k].rearrange("j p f -> p j f"))
                    ye_ps = ps_big.tile([128, N], F32, tag="acc")
                    for j in range(MT):
                        jsl = slice(j * 128, (j + 1) * 128)
                        jsl2 = slice(MT * 128 + j * 128,
                                     MT * 128 + (j + 1) * 128)
                        hhj = hh[:, j * N:(j + 1) * N]
                        hlj = hl[:, j * N:(j + 1) * N]
                        nc.tensor.matmul(ye_ps[:], w2k[:, jsl], hhj,
                                         start=(j == 0), stop=False)
                        nc.tensor.matmul(ye_ps[:], w2k[:, jsl], hlj,
                                         start=False, stop=False)
                        nc.tensor.matmul(ye_ps[:], w2k[:, jsl2], hhj,
                                         start=False, stop=(j == MT - 1))
                    ysc = sm.tile([128, N], F32, tag="ycp", bufs=2)
                    nc.vector.tensor_mul(ysc[:], ye_ps[:], cbc[:])
                    nc.sync.dma_start(ar_moe_in[l][k][:], ysc[:])
                    nc.gpsimd.collective_compute(
                        "AllReduce", ALU.add,
                        replica_groups=[list(range(N_CORES))],
                        ins=[ar_moe_in[l][k][:]], outs=[ar_moe_out[l][k][:]])

                # LN2: consume per-k AR results as they land (in place on x)
                s_ps = ps_small.tile([1, N], F32, tag="sm")
                q_ps = ps_small.tile([1, N], F32, tag="sm")
                for k in range(KT):
                    yk = wk.tile([128, N], F32, name="yk", bufs=2)
                    nc.sync.dma_start(yk[:], ar_moe_out[l][k][:])
                    b2c_ps = ps_bc.tile([128, N], F32, tag="bc")
                    nc.tensor.matmul(b2c_ps[:], b2t[:, k * 128:(k + 1) * 128],
                                     combT[:], start=True, stop=True)
                    nc.vector.tensor_add(yk[:], yk[:], b2c_ps[:])
                    nc.vector.tensor_add(xs(k), xs(k), yk[:])
                    sq = sm.tile([128, N], F32, tag="lnsq", bufs=2)
                    nc.scalar.activation(sq[:], xs(k), AF.Square)
                    nc.tensor.matmul(s_ps[:], ones_col[:], xs(k),
                                     start=(k == 0), stop=(k == KT - 1))
                    nc.tensor.matmul(q_ps[:], ones_col[:], sq[:],
                                     start=(k == 0), stop=(k == KT - 1))
                layernorm_cols(slice(0, N), l2w, l2b,
                               presummed=True, s_ps=s_ps, q_ps=q_ps)

            # ---- final ln + lm head ----
            layernorm_cols(slice(0, N), lnfw, lnfb)
            lg_ps = ps_big.tile([V, N], F32, tag="acc")
            for k in range(KT):
                nc.tensor.matmul(lg_ps[:], lmT[:, k * V:(k + 1) * V], xs(k),
                                 start=(k == 0), stop=(k == KT - 1))
            lg = sm.tile([V, N], F32, tag="lgout", bufs=1)
            nc.scalar.activation(lg[:], lg_ps[:], AF.Identity, bias=lmb[:])
            nc.sync.dma_start(d_out[:], lg[:])

    return nc


def _bf_split(a):
    """bf16 hi/lo split of an fp32 array."""
    a = np.ascontiguousarray(a, dtype=np.float32)
    hi = a.astype(ml_dtypes.bfloat16)
    lo = (a - hi.astype(np.float32)).astype(ml_dtypes.bfloat16)
    return np.ascontiguousarray(hi), np.ascontiguousarray(lo)


def _prep(inputs):
    """Build per-core input maps from the full input dict."""
    f = lambda a: np.ascontiguousarray(np.asarray(a), dtype=np.float32)
    idx = np.asarray(inputs["idx"]).reshape(1, N)
    wq, wkk, wv = f(inputs["wq"]), f(inputs["wk"]), f(inputs["wv"])
    wproj, bproj = f(inputs["wproj"]), f(inputs["bproj"])
    gate_w, gate_b = f(inputs["gate_w"]), f(inputs["gate_b"])
    w1, b1 = f(inputs["w1"]), f(inputs["b1"])
    w2, b2 = f(inputs["w2"]), f(inputs["b2"])

    base = {
        "idx": np.ascontiguousarray(idx.astype(np.int32)),
        "iota99": np.arange(V, dtype=np.float32).reshape(V, 1),
        "ident128": np.eye(128, dtype=np.float32),
        "maskb": np.where(np.tril(np.ones((64, 64), bool)), 0.0,
                          -1e30).astype(np.float32),
        "ones_col": np.ones((128, 1), np.float32),
        "ones_row": np.ones((1, 128), np.float32),
        "tok_emb": f(inputs["tok_emb"]),
        "posT": np.ascontiguousarray(
            np.tile(f(inputs["pos_emb"]).T, (1, B))),
        "gwT": np.ascontiguousarray(
            gate_w.transpose(0, 2, 1).reshape(L, KT, 128, E)),
        "gb": gate_b.reshape(L, 1, E),
        "b2all": b2,
        "ln1w": f(inputs["ln1_w"]).reshape(L, KT, 128),
        "ln1b": f(inputs["ln1_b"]).reshape(L, KT, 128),
        "ln2w": f(inputs["ln2_w"]).reshape(L, KT, 128),
        "ln2b": f(inputs["ln2_b"]).reshape(L, KT, 128),
        "lnfw": f(inputs["lnf_w"]).reshape(KT, 128),
        "lnfb": f(inputs["lnf_b"]).reshape(KT, 128),
        "lmT": np.ascontiguousarray(f(inputs["lm_w"]).T.reshape(KT, 128, V)),
        "lmb": f(inputs["lm_b"]).reshape(V, 1),
        "bproj": bproj.reshape(L, KT, 128),
    }
    in_maps = []
    for c in range(N_CORES):
        m = dict(base)
        wqT = np.ascontiguousarray(
            wq[:, c].transpose(0, 2, 1).reshape(L, KT, 128, H))
        wkT = np.ascontiguousarray(
            wkk[:, c].transpose(0, 2, 1).reshape(L, KT, 128, H))
        wvT = np.ascontiguousarray(
            wv[:, c].transpose(0, 2, 1).reshape(L, KT, 128, H))
        m["wqh"], m["wql"] = _bf_split(wqT)
        m["wkh"], m["wkl"] = _bf_split(wkT)
        m["wvh"], m["wvl"] = _bf_split(wvT)
        wpT = np.ascontiguousarray(
            wproj[:, :, c * H:(c + 1) * H].transpose(0, 2, 1))
        m["wph"], m["wpl"] = _bf_split(wpT)
        w1tc = w1[:, c].transpose(0, 2, 1)  # [L, 768, 3072]
        w1T = np.ascontiguousarray(
            w1tc.reshape(L, KT, 128, MT, 128).transpose(0, 3, 1, 2, 4))
        m["w1h"], m["w1l"] = _bf_split(w1T)
        m["b1"] = np.ascontiguousarray(b1[:, c].reshape(L, MT, 128))
        w2tc = w2[:, c].transpose(0, 2, 1)  # [L, 3072, 768]
        w2T = np.ascontiguousarray(
            w2tc.reshape(L, MT, 128, KT, 128).transpose(0, 3, 1, 2, 4))
        m["w2h"], m["w2l"] = _bf_split(w2T)
        sel = np.zeros((E, 1), np.float32)
        sel[c, 0] = 1.0
        m["combsel"] = sel
        in_maps.append(m)
    return in_maps


def kernel(**inputs) -> np.ndarray:
    if "nc" not in _CACHED:
        _CACHED["nc"] = build()
    nc = _CACHED["nc"]
    in_maps = _prep(inputs)
    res = run_bass_kernel_spmd(nc, in_maps, list(range(N_CORES)))
    lt = res.results[0]["logitsT"]  # [V, N]
    return np.ascontiguousarray(lt.T.reshape(B, T, V).astype(np.float32))


if __name__ == "__main__":
    import jax

    jax.config.update("jax_platforms", "cpu")
    import reference as ref

    inp = ref.setup_inputs()
    want = np.asarray(ref.reference(**inp))
    import jax as _j
    _j.config.update("jax_platforms", "axon")
    got = kernel(**{k: np.asarray(v) for k, v in inp.items()})
    err = np.abs(got - want).max()
    rel = err / np.abs(want).max()
    l2 = np.linalg.norm(got - want) / np.linalg.norm(want)
    print(f"absmax {err:.3e}  absmax-rel {rel:.3e}  l2-rel {l2:.3e}")
